# revision 18
# baseline (speedup 1.0000x reference)
"""Trainium2 Bass kernel for nn_ActQuantWrapper (hadamard + per-token act quant + linear).

Math (per reference):
  z = (H_64 kron I_had) x / 8               -- FHT over 64 groups along feature dim
  sx[t] = clip(absmax(z[t,:])/127, 1e-5)    -- per-token scale
  xq = round(z/sx)*sx                        -- act quant-dequant
  out = xq @ weight.T + bias                 -- weight already per-channel quantized

Device strategy (8 cores, data-parallel over tokens, weight replicated):
  - qx = round(z/sx) are integers in [-127,127]: exactly representable in
    fp16, so the x operand is lossless; the psum is scaled by sx[t] after.
  - weight/x/bias are staged host-side in fp16 (weight also pre-transposed
    into the k-major SBUF tile layout), so the device does no weight-side
    work and the FHT butterflies run in fp16 on DVE (2x perf mode).
  - per-token absmax runs on GpSimd to keep DVE free for the FHT.
  - activation rounding uses the fp32 magic-number trick on ACT + DVE.
  - epilogue fuses to a single op: out = psum * sx[t] + bias.
  - matmul groups are emitted in waves over the first WAVE_W weight chunks
    so the PE consumes token tiles as the x-path produces them; after the
    first matmul the PE runs gap-free at the fp16 roofline.
"""

import numpy as np

import concourse.bass as bass
import concourse.tile as tile
from concourse import bacc, mybir
from concourse.bass_utils import run_bass_kernel_spmd

F32 = mybir.dt.float32
F16 = mybir.dt.float16
MAGIC = 1536.0  # 1.5 * 2**10: fp16 ulp is exactly 1.0 in [1024, 2048)

N_CORES = 8
B, S, D_IN, D_OUT = 2, 2048, 4096, 4096
N_TOK = B * S
T_CORE = N_TOK // N_CORES  # 512 tokens per core
N_GROUPS = 64              # hadamard dimension (fixed by reference)
OC_SIZE = 512              # output-chunk width (one PSUM bank)
WAVE_W = 3                 # weight chunks consumed wave-interleaved at start


def build_kernel(n_tok, K, O, oc_size, trace_sim=False):
    assert n_tok % 128 == 0 and K % 256 == 0 and O % oc_size == 0
    n_tt = n_tok // 128     # token tiles
    n_kt = K // 128         # contraction tiles
    n_oc = O // oc_size     # output chunks
    had_dim = K // N_GROUPS

    nc = bacc.Bacc("TRN2", target_bir_lowering=False, debug=False)
    x_d = nc.dram_tensor("x", [n_tok, K], F16, kind="ExternalInput")
    # weight pre-transposed+tiled on host: [n_oc*128, n_kt*oc_size] fp16 where
    # row (oc*128 + p), col (kb*oc_size + c) holds weight[oc*oc_size + c, kb*128 + p]
    wt_d = nc.dram_tensor("wt", [n_oc * 128, n_kt * oc_size], F16,
                          kind="ExternalInput")
    b_d = nc.dram_tensor("b", [O], F16, kind="ExternalInput")
    out_d = nc.dram_tensor("out", [n_tok, O], F32, kind="ExternalOutput")

    with tile.TileContext(nc, trace_sim=trace_sim) as tc:
        with (
            tc.tile_pool(name="xload", bufs=2) as xload,
            tc.tile_pool(name="xwork", bufs=1) as xwork,
            tc.tile_pool(name="qtp", bufs=1) as qtp,
            tc.tile_pool(name="qxp", bufs=1) as qxp,
            tc.tile_pool(name="wload", bufs=WAVE_W) as wload,
            tc.tile_pool(name="outp", bufs=2) as outp,
            tc.tile_pool(name="consts", bufs=1) as consts,
            tc.tile_pool(name="psum", bufs=1, space=bass.MemorySpace.PSUM) as psum,
        ):
            qxT = consts.tile([128, n_kt, n_tok], F16)
            sx_all = consts.tile([128, n_tt], F32)
            xsc = consts.tile([128, n_tt, 3], F32)  # m, r, r8 per token tile
            bb_all = consts.tile([128, n_oc, oc_size], F16)  # bias broadcasts

            # ---------------- x path: FHT -> quant -> transpose ----------------
            for tt in range(n_tt):
                za = xload.tile([128, K], F16, tag="za")
                nc.sync.dma_start(za[:], x_d.ap()[tt * 128:(tt + 1) * 128, :])
                zb = xwork.tile([128, K], F16, tag="zb")
                bufs = [za, zb]
                for s in range(6):
                    src, dst = bufs[s % 2], bufs[(s + 1) % 2]
                    blk = had_dim << s
                    sv = src[:].rearrange("p (a c b) -> p a c b", c=2, b=blk)
                    dv = dst[:].rearrange("p (a c b) -> p a c b", c=2, b=blk)
                    nc.vector.tensor_add(dv[:, :, 0, :], sv[:, :, 0, :], sv[:, :, 1, :])
                    nc.vector.tensor_sub(dv[:, :, 1, :], sv[:, :, 0, :], sv[:, :, 1, :])
                # 6 stages end back in za (unscaled by 1/8; folded into the scale)
                m = xsc[:, tt, 0:1]
                nc.vector.tensor_reduce(
                    out=m, in_=za[:], axis=mybir.AxisListType.X,
                    op=mybir.AluOpType.max, apply_absolute_value=True,
                )
                # sx = clip((m/8)/127, 1e-5) = clip(m/1016, 1e-5); m/8 is exact
                nc.vector.tensor_scalar(
                    out=sx_all[:, tt:tt + 1], in0=m,
                    scalar1=float(np.float32(1.0) / np.float32(1016.0)),
                    scalar2=1e-5,
                    op0=mybir.AluOpType.mult, op1=mybir.AluOpType.max,
                )
                rx = xsc[:, tt, 1:2]
                nc.vector.reciprocal(rx, sx_all[:, tt:tt + 1])
                rx8 = xsc[:, tt, 2:3]
                nc.vector.tensor_scalar_mul(rx8, rx, 0.125)
                # v = za*rx8 = z/sx in [-127.5, 127.5]; v + 1536 lies in
                # [1024, 2048) where fp16 ulp is exactly 1.0, so the fp16
                # output conversion rounds v to the nearest integer (RNE),
                # matching jnp.round. ACT computes in fp32 and rounds once
                # at the fp16 write; the subtract runs at DVE 4x fp16 mode.
                qtmp = qtp.tile([128, K], F16, tag="qt")
                nc.scalar.activation(
                    out=qtmp[:], in_=za[:], func=mybir.ActivationFunctionType.Copy,
                    bias=MAGIC, scale=rx8,
                )
                qx = qxp.tile([128, K], F16, tag="qx")
                nc.vector.tensor_scalar_add(qx[:], qtmp[:], -MAGIC)
                nc.scalar.dma_start_transpose(
                    qxT[:, :, tt * 128:(tt + 1) * 128], qx[:]
                )

            # ---------------- weight load + matmul ----------------
            qwts = {}

            def load_chunk(oc):
                qwT = wload.tile([128, n_kt, oc_size], F16, tag="qwT")
                nc.sync.dma_start(
                    qwT[:].rearrange("p a b -> p (a b)"),
                    wt_d.ap()[oc * 128:(oc + 1) * 128, :],
                )
                srcb = b_d.ap()[oc * oc_size:(oc + 1) * oc_size]
                nc.gpsimd.dma_start(
                    out=bb_all[:, oc, :],
                    in_=bass.AP(tensor=srcb.tensor, offset=srcb.offset,
                                ap=[[0, 128]] + list(srcb.ap)),
                )
                qwts[oc] = qwT

            gi = 0

            def group(oc, t):
                nonlocal gi
                qwT = qwts[oc]
                ps = psum.tile([128, oc_size], F32, tag=f"ps{gi % 8}")
                gi += 1
                for k in range(n_kt):
                    nc.tensor.matmul(
                        ps[:],
                        qxT[:, k, t * 128:(t + 1) * 128],
                        qwT[:, k, :],
                        start=(k == 0), stop=(k == n_kt - 1),
                    )
                o_sb = outp.tile([128, oc_size], F32, tag="osb")
                # out = psum * sx[t] + bias
                nc.vector.scalar_tensor_tensor(
                    out=o_sb[:], in0=ps[:], scalar=sx_all[:, t:t + 1],
                    in1=bb_all[:, oc, :],
                    op0=mybir.AluOpType.mult, op1=mybir.AluOpType.add,
                )
                nc.gpsimd.dma_start(
                    out_d.ap()[t * 128:(t + 1) * 128,
                               oc * oc_size:(oc + 1) * oc_size],
                    o_sb[:],
                )

            # wave phase: first WAVE_W chunks consume token tiles as produced
            for oc in range(WAVE_W):
                load_chunk(oc)
            for t in range(n_tt):
                for oc in range(WAVE_W):
                    group(oc, t)
            # steady phase: remaining chunks run all token tiles back-to-back
            for oc in range(WAVE_W, n_oc):
                load_chunk(oc)
                for t in range(n_tt):
                    group(oc, t)

    nc.compile()
    return nc


_CACHED = None


def _get_full_kernel():
    global _CACHED
    if _CACHED is None:
        _CACHED = build_kernel(T_CORE, D_IN, D_OUT, OC_SIZE)
    return _CACHED


def prep_weight(weight):
    """Host-side: fp16-cast + retile weight into the layout wt_d expects."""
    n_oc = D_OUT // OC_SIZE
    n_kt = D_IN // 128
    w = np.asarray(weight, dtype=np.float32)
    # [oc, c, kb, p] -> [oc, p, kb, c]
    wt = w.reshape(n_oc, OC_SIZE, n_kt, 128).transpose(0, 3, 2, 1)
    wt = np.ascontiguousarray(wt).astype(np.float16)
    return wt.reshape(n_oc * 128, n_kt * OC_SIZE)


def make_in_maps(x, weight, bias):
    xf = np.asarray(x).reshape(N_TOK, D_IN).astype(np.float16)
    xf = np.ascontiguousarray(xf)
    wt = prep_weight(weight)
    bi = np.ascontiguousarray(np.asarray(bias).astype(np.float16))
    return [
        {"x": xf[i * T_CORE:(i + 1) * T_CORE], "wt": wt, "b": bi}
        for i in range(N_CORES)
    ]


def kernel(x, weight, bias, had_dim):
    assert int(had_dim) == 64
    assert x.shape == (B, S, D_IN) and weight.shape == (D_OUT, D_IN)
    nc = _get_full_kernel()
    in_maps = make_in_maps(x, weight, bias)
    res = run_bass_kernel_spmd(nc, in_maps, core_ids=list(range(N_CORES)))
    out = np.concatenate([r["out"] for r in res.results], axis=0)
    return out.reshape(B, S, D_OUT)


if __name__ == "__main__":
    rng = np.random.default_rng(0)
    x = rng.standard_normal((B, S, D_IN), dtype=np.float32)
    w = rng.standard_normal((D_OUT, D_IN), dtype=np.float32)
    b = rng.standard_normal(D_OUT).astype(np.float32)
    o = kernel(x, w, b, np.int64(64))
    print(o.shape, o.dtype)
